# revision 1
# baseline (speedup 1.0000x reference)
"""nn_Attention: out[b,h] = strict_tril(rope(Q[b,h]) @ rope(Q[b,h])^T) @ V[b].

Sharding: one (b,h) pair per NeuronCore (B*H = 8 pairs on 8 cores, fully
data-parallel, no collectives).  Host-side staging de-interleaves Q's
even/odd columns (a pure relayout: scores contract over all of n, so any
fixed n-permutation is mathematically neutral), which makes every RoPE
elementwise op dense step-1 bf16 (DVE 2x perf mode) against half-size
per-pair cos/sin tables.

Per core, emitted in 8 "waves" of 256 rows so the tensor engine starts as
soon as the first tiles are ready:

  phase 0 : Q row tiles f32->bf16 cast-load (SWDGE, 4 queues), per-pair
            cos/sin tables on the ACT HWDGE ring; RoPE on DVE:
              A = [Qe|Qo]*c, B = [Qe|Qo]*s (doubled-table broadcast APs),
              QRe = A_e - B_o, QRo = A_o + B_e.
  phase 0b: transpose QR row tiles into QR^T chunk layout with PE
            transpose-mode (8 blocks per PSUM bank, ACT copies the bank
            into the chunk columns) - cheaper than the DMA xbar, and the
            data lands right after RoPE with no ring latency.
  phase A : score strips T_j = QR_j @ QR^T[:, jP:] (lower-triangle blocks
            only; scores are symmetric so T_ji doubles as the transposed
            lhsT for phase B), bf16 matmuls accumulated in f32 PSUM,
            strict-upper mask on the diagonal block, cast to bf16 strips.
  phase B : out_i = sum_{j<=i} matmul(lhsT=T_ji, rhs=V_j) accumulated in
            PSUM, copied out as f32 and stored per row block.
"""

import math
from functools import lru_cache

import numpy as np
import ml_dtypes

import concourse.bass as bass
import concourse.mybir as mybir
import concourse.tile as tile
from concourse import bacc
from concourse.bass_utils import run_bass_kernel_spmd
from concourse.masks import make_upper_triangular

THETA = 2.0 ** 16
P = 128
TMODE = "pe"  # transpose strategy: "hbm" | "sbuf2d" | "sbuf3d" | "pe"

BF16 = mybir.dt.bfloat16
F32 = mybir.dt.float32


@lru_cache(maxsize=None)
def _rope_tables(t, n):
    """cos/sin tables matching reference._rope, bf16, sin sign-baked.

    QR[:,2p]   = q[2p]*c_p - q[2p+1]*s_p
    QR[:,2p+1] = q[2p+1]*c_p + q[2p]*s_p
    With A = q*cos, Bm = q*sinm where sinm[:,2p]=+s_p, sinm[:,2p+1]=-s_p:
      QR[:,even] = A[:,even] + Bm[:,odd]
      QR[:,odd]  = A[:,odd]  + Bm[:,even]
    """
    idx = ((np.arange(n) // 2) * 2).astype(np.float32)
    freqs = (1.0 / (THETA ** (idx / np.float32(n))) / np.float32(2.0 * math.pi)).astype(
        np.float32
    )
    pos = np.arange(t, dtype=np.float32)[:, None]
    phases = ((pos * freqs) % np.float32(1.0)) * np.float32(2.0 * math.pi)
    # one table entry per pair (reference quantizes freqs in pairs)
    cos_h = np.cos(phases)[:, 0::2]
    sin_h = np.sin(phases)[:, 0::2]
    return (
        np.ascontiguousarray(cos_h.astype(ml_dtypes.bfloat16)),
        np.ascontiguousarray(sin_h.astype(ml_dtypes.bfloat16)),
    )


@lru_cache(maxsize=None)
def _build(t, n, d, tmode="pe", ww=256):
    from contextlib import ExitStack

    nt = t // P      # row blocks
    nk = n // P      # contraction chunks
    tq = min(ww, t)  # rows per wave == score psum chunk width
    nw = t // tq     # number of waves
    ltw = tq // P    # row tiles per wave
    assert tq <= 512 and tq % P == 0 and n % P == 0 and t % tq == 0

    nc = bacc.Bacc("TRN2", target_bir_lowering=False, debug=False, num_swdge_queues=4)
    q_d = nc.dram_tensor("q", [t, n], F32, kind="ExternalInput").ap()
    v_d = nc.dram_tensor("v", [t, d], F32, kind="ExternalInput").ap()
    cos_d = nc.dram_tensor("cos_t", [t, n // 2], BF16, kind="ExternalInput").ap()
    sin_d = nc.dram_tensor("sin_t", [t, n // 2], BF16, kind="ExternalInput").ap()
    out_d = nc.dram_tensor("out", [t, d], F32, kind="ExternalOutput").ap()

    with tile.TileContext(nc) as tc, ExitStack() as ctx:
        if tmode == "hbm":
            dram = ctx.enter_context(tc.tile_pool(name="dram", bufs=1, space="DRAM"))
            qr_hbm = [dram.tile([tq, n], BF16, name=f"qr_hbm_{w}") for w in range(nw)]

        const = ctx.enter_context(tc.tile_pool(name="const", bufs=1))
        umask = const.tile([P, P], BF16, name="umask")
        if tmode == "pe":
            ident = const.tile([P, P], BF16, name="ident")
            from concourse.masks import make_identity
            make_identity(nc, ident)

        vpool = ctx.enter_context(tc.tile_pool(name="vpool", bufs=1))
        vb = vpool.tile([P, nt * d], BF16, name="vb")

        qrt_pool = ctx.enter_context(tc.tile_pool(name="qrt_pool", bufs=1))
        # QR^T: chunk k ([n in [kP,(k+1)P)] x [t]) lives at cols [k*t,(k+1)*t)
        qrt = qrt_pool.tile([P, nk * t], BF16, name="qrt")

        strips_pool = ctx.enter_context(tc.tile_pool(name="strips", bufs=1))
        # strip j = T_j,(j..nt) = QR_j @ QR^T[:, jP:] as [s(128) x t(width)]
        strips = [
            strips_pool.tile([P, (nt - j) * P], BF16, name=f"strip{j}")
            for j in range(nt)
        ]

        qpool = ctx.enter_context(tc.tile_pool(name="qpool", bufs=3))
        cpool = ctx.enter_context(tc.tile_pool(name="cpool", bufs=3))
        spool = ctx.enter_context(tc.tile_pool(name="spool", bufs=3))
        apool = ctx.enter_context(tc.tile_pool(name="apool", bufs=3))
        bpool = ctx.enter_context(tc.tile_pool(name="bpool", bufs=3))
        rpool = ctx.enter_context(tc.tile_pool(name="rpool", bufs=3))
        outp = ctx.enter_context(tc.tile_pool(name="outp", bufs=3))

        if tmode == "pe":
            spsum = ctx.enter_context(tc.tile_pool(name="spsum", bufs=5, space="PSUM"))
            opsum = ctx.enter_context(tc.tile_pool(name="opsum", bufs=1, space="PSUM"))
            tpsum = ctx.enter_context(tc.tile_pool(name="tpsum", bufs=2, space="PSUM"))
        else:
            spsum = ctx.enter_context(tc.tile_pool(name="spsum", bufs=6, space="PSUM"))
            opsum = ctx.enter_context(tc.tile_pool(name="opsum", bufs=2, space="PSUM"))

        for w in range(nw):
            # ---- phase 0: RoPE the row tiles of wave w -------------------
            # batched loads: one DMA per wave covers all its row tiles
            r0w = w * tq
            qb2 = qpool.tile([P, ltw, n], BF16, tag="qb", name=f"qb_{w}")
            ct2 = cpool.tile([P, ltw, n // 2], BF16, tag="ct", name=f"ct_{w}")
            st2 = spool.tile([P, ltw, n // 2], BF16, tag="st", name=f"st_{w}")
            if w == 0:
                # latency-optimized first wave: per-tile loads so tile 0's
                # RoPE starts as early as possible
                for lt in range(ltw):
                    r0 = r0w + lt * P
                    nc.gpsimd.dma_start(
                        out=qb2[:, lt, :], in_=q_d[r0 : r0 + P, :]
                    )
                    nc.scalar.dma_start(
                        out=ct2[:, lt, :], in_=cos_d[r0 : r0 + P, :]
                    )
                    nc.scalar.dma_start(
                        out=st2[:, lt, :], in_=sin_d[r0 : r0 + P, :]
                    )
                make_upper_triangular(nc, umask, val=1.0, diag=False)
            else:
                nc.gpsimd.dma_start(
                    out=qb2,
                    in_=q_d[r0w : r0w + tq, :].rearrange("(s p) c -> p s c", p=P),
                )
                nc.scalar.dma_start(
                    out=ct2,
                    in_=cos_d[r0w : r0w + tq, :].rearrange("(s p) c -> p s c", p=P),
                )
                nc.scalar.dma_start(
                    out=st2,
                    in_=sin_d[r0w : r0w + tq, :].rearrange("(s p) c -> p s c", p=P),
                )
            if w == 0:
                # V is first needed by phase B of wave 0; keep it off the
                # head of the SWDGE FIFO so wave-0 Q loads start immediately
                nc.gpsimd.dma_start(
                    out=vb.rearrange("p (j dd) -> p j dd", j=nt),
                    in_=v_d.rearrange("(j p) dd -> p j dd", p=P),
                )
            for lt in range(ltw):
                r0 = w * tq + lt * P
                qb = qb2[:, lt, :]
                ct = ct2[:, lt, :]
                st = st2[:, lt, :]

                # Q arrives column-de-interleaved from the host: [Qe | Qo].
                # A = [Qe*c | Qo*c], B = [Qe*s | Qo*s] via a doubled-table
                # broadcast AP (innermost step 1 -> DVE 2x eligible), then
                # QRe = A_e - B_o, QRo = A_o + B_e, all dense slices.
                hn = n // 2
                a_t = apool.tile([P, n], BF16, tag="a", name=f"a_{w}_{lt}")
                b_t = bpool.tile([P, n], BF16, tag="b", name=f"b_{w}_{lt}")
                qb3 = qb.rearrange("p (two k) -> p two k", two=2)
                ct_b = ct.unsqueeze(1).broadcast_to([P, 2, hn])
                st_b = st.unsqueeze(1).broadcast_to([P, 2, hn])
                nc.vector.tensor_mul(
                    a_t.rearrange("p (two k) -> p two k", two=2), qb3, ct_b
                )
                nc.vector.tensor_mul(
                    b_t.rearrange("p (two k) -> p two k", two=2), qb3, st_b
                )
                qr_t = rpool.tile([P, n], BF16, tag="qr", name=f"qr_{w}_{lt}")
                nc.vector.tensor_sub(qr_t[:, 0:hn], a_t[:, 0:hn], b_t[:, hn:n])
                nc.vector.tensor_add(qr_t[:, hn:n], a_t[:, hn:n], b_t[:, 0:hn])

                it = w * ltw + lt
                if tmode == "hbm":
                    nc.gpsimd.dma_start(
                        out=qr_hbm[w][lt * P : (lt + 1) * P, :], in_=qr_t
                    )
                elif tmode == "sbuf2d":
                    # per-chunk SBUF->SBUF xbar transposes of this row tile
                    for k in range(nk):
                        nc.sync.dma_start(
                            out=qrt[:, k * t + it * P : k * t + (it + 1) * P],
                            in_=qr_t[:, k * P : (k + 1) * P],
                            transpose=True,
                        )
                elif tmode == "pe":
                    # PE transpose-mode, 8 chunk blocks per PSUM bank, ACT
                    # copies the bank into the chunk columns of qrt
                    qrt3 = qrt.rearrange("p (k tl) -> p k tl", k=nk)
                    for k0 in range(0, nk, 8):
                        nb = min(8, nk - k0)
                        tp = tpsum.tile(
                            [P, nb * P], BF16, tag="tp", name=f"tp_{it}_{k0}"
                        )
                        for b in range(nb):
                            k = k0 + b
                            nc.tensor.transpose(
                                tp[:, b * P : (b + 1) * P],
                                qr_t[:, k * P : (k + 1) * P],
                                ident,
                            )
                        nc.scalar.copy(
                            qrt3[:, k0 : k0 + nb, it * P : (it + 1) * P],
                            tp.rearrange("p (b tl) -> p b tl", b=nb),
                        )
                elif tmode == "sbuf3d":
                    # one xbar transpose per row tile; rows of qr_t^T are
                    # scattered over (partition, chunk) in whatever bijective
                    # order the xbar uses — harmless, since both matmul
                    # operands index chunks through the same qrt layout.
                    nc.sync.dma_start(
                        out=qrt.rearrange("p (k tl) -> p k tl", k=nk)[
                            :, :, it * P : (it + 1) * P
                        ],
                        in_=qr_t,
                        transpose=True,
                    )
                else:
                    raise ValueError(tmode)

            # ---- phase 0b: transpose-load wave w of every chunk ----------
            if tmode == "hbm":
                for k in range(nk):
                    nc.sync.dma_start(
                        out=qrt[:, k * t + w * tq : k * t + (w + 1) * tq],
                        in_=qr_hbm[w][:, k * P : (k + 1) * P],
                        transpose=True,
                    )

            # ---- phase A: score strip chunks landing in wave w -----------
            # wave 0 uses single-block chunks so the PE can start as soon as
            # the first row tile is transposed
            sub = P if w == 0 else tq
            for j in range((w + 1) * tq // P):
                for lo in range(max(j * P, w * tq), (w + 1) * tq, sub):
                    hi = min((w + 1) * tq, lo + sub)
                    width = hi - lo
                    ps = spsum.tile(
                        [P, width], F32, tag="ps", name=f"ps_{w}_{j}_{lo}"
                    )
                    for k in range(nk):
                        nc.tensor.matmul(
                            ps,
                            lhsT=qrt[:, k * t + j * P : k * t + (j + 1) * P],
                            rhs=qrt[:, k * t + lo : k * t + hi],
                            start=(k == 0),
                            stop=(k == nk - 1),
                        )
                    l0 = lo - j * P
                    if l0 == 0:
                        # diagonal block: strict upper triangle in [s,t]
                        nc.vector.tensor_mul(
                            strips[j][:, 0:P], ps[:, 0:P], umask
                        )
                        if width > P:
                            nc.scalar.copy(
                                strips[j][:, P:width], ps[:, P:width]
                            )
                    else:
                        nc.scalar.copy(
                            strips[j][:, l0 : l0 + width], ps[:, :width]
                        )

            # ---- phase B: outputs for row blocks of wave w ---------------
            for i in range(w * tq // P, (w + 1) * tq // P):
                po = opsum.tile([P, d], F32, tag="po", name=f"po_{i}")
                for jj in range(i + 1):
                    nc.tensor.matmul(
                        po,
                        lhsT=strips[jj][:, (i - jj) * P : (i - jj + 1) * P],
                        rhs=vb[:, jj * d : (jj + 1) * d],
                        start=(jj == 0),
                        stop=(jj == i),
                    )
                ot = outp.tile([P, d], F32, tag="ot", name=f"ot_{i}")
                nc.scalar.copy(ot, po)
                nc.scalar.dma_start(out=out_d[i * P : (i + 1) * P, :], in_=ot)

    nc.compile()
    return nc


def _run(Q, V, trace=False, **trace_kwargs):
    Q = np.asarray(Q, dtype=np.float32)
    V = np.asarray(V, dtype=np.float32)
    b, h, t, n = Q.shape
    d = V.shape[-1]
    ncores = b * h
    nc = _build(t, n, d, tmode=TMODE)
    cos_t, sin_t = _rope_tables(t, n)
    in_maps = []
    for core in range(ncores):
        bi, hi = divmod(core, h)
        qde = np.empty((t, n), dtype=np.float32)
        qde[:, : n // 2] = Q[bi, hi][:, 0::2]
        qde[:, n // 2 :] = Q[bi, hi][:, 1::2]
        in_maps.append(
            {
                "q": qde,
                "v": np.ascontiguousarray(V[bi, 0]),
                "cos_t": cos_t,
                "sin_t": sin_t,
            }
        )
    res = run_bass_kernel_spmd(
        nc, in_maps, core_ids=list(range(ncores)), trace=trace, **trace_kwargs
    )
    out = np.empty((b, h, t, d), dtype=np.float32)
    for core in range(ncores):
        bi, hi = divmod(core, h)
        out[bi, hi] = res.results[core]["out"]
    return out, res


def kernel(**inputs):
    out, _ = _run(inputs["Q"], inputs["V"], trace=False)
    return out

